# revision 63
# baseline (speedup 1.0000x reference)
"""Multi-head attention (B=1, S=4096, D=1024, H=16, Hd=64) on 8 Trainium2 cores.

Sharding: tensor-parallel over heads - 2 heads per core. Each core computes
q/k/v projections for its 2 heads (128 dims), flash-style attention without
max-subtraction (scores are ~N(0,1) after scaling so exp never overflows),
and a partial output projection with its 128 rows of wo. Host sums the 8
partial outputs and adds bo.

v2 design (ACT-roofline targeted):
  - k/q/v and the exp'd scores are bf16 on SBUF: score matmuls get FWL
    weight loads and the two K=64 head-matmuls are explicitly row-tiled
    (tile_position (0,0)/(64,0)) into different PSUM banks so they run
    concurrently in the PE array.
  - PSUM: 4 banks score staging (2 bufs x [128,2,512]), 2 banks ctx
    accumulators (per-head, 65 rows: 64 ctx dims + ones-column denominator),
    2 banks aux ring used by projection accumulation first and by the
    normalize-broadcast + out-projection afterwards, so the per-q-block
    epilogue never blocks the next q-block's accumulation.
  - V is transposed to [k, hd] layout by the DMA xbar (bf16) instead of PE
    transposes.
  - Projections for x-blocks 1..7 are software-pipelined into q-block 0's
    attention groups.
  - Normalizer broadcast is one K=2 masked matmul for both heads.
"""

import os
import sys
import types

import numpy as np

S = 4096
D = 1024
H = 16
HD = 64
N_CORES = 8
HPC = H // N_CORES  # heads per core = 2
DC = D // 128       # d-chunks = 8
QB = 512            # q block
KC = S // 128       # k chunks = 32

_LAST_EXEC_NS = None


def _install_ntff_hook_shim():
    if "antenv.axon_hooks" in sys.modules:
        return
    try:
        import antenv
        from trn_agent_boot.trn_boot import _ntff_profile_via_ctypes

        hook = _ntff_profile_via_ctypes("/opt/axon/libaxon_pjrt.so")
    except Exception:
        return
    mod = types.ModuleType("antenv.axon_hooks")
    _state = {"hook": hook}
    mod.get_axon_ntff_profile_hook = lambda: _state["hook"]
    mod.set_axon_ntff_profile_hook = lambda h: _state.update(hook=h)
    sys.modules["antenv.axon_hooks"] = mod
    antenv.axon_hooks = mod


def _build(s=S, debug=False):
    import concourse.bass as bass
    import concourse.mybir as mybir
    import concourse.tile as tile
    from concourse import bacc
    from concourse.masks import make_identity

    f32 = mybir.dt.float32
    f32r = mybir.dt.float32r
    bf16 = mybir.dt.bfloat16
    Exp = mybir.ActivationFunctionType.Exp

    kc_total = s // 128
    NP = s // QB            # x/q blocks = 8
    NKP = kc_total // 2     # kc pairs per q block = 16

    nc = bacc.Bacc("TRN2", target_bir_lowering=False, debug=False,
                   num_devices=N_CORES)

    xT_d = nc.declare_dram_parameter("xT", [NP, 128, DC, QB], bf16,
                                     isOutput=False)
    wq_d = nc.declare_dram_parameter("wq", [128, D], bf16, isOutput=False)
    wk_d = nc.declare_dram_parameter("wk", [128, D], bf16, isOutput=False)
    wv_d = nc.declare_dram_parameter("wv", [128, D], bf16, isOutput=False)
    bq_d = nc.declare_dram_parameter("bq", [128, 1], f32, isOutput=False)
    bk_d = nc.declare_dram_parameter("bk", [128, 1], f32, isOutput=False)
    bv_d = nc.declare_dram_parameter("bv", [128, 1], f32, isOutput=False)
    wo_d = nc.declare_dram_parameter("wo", [128, D], f32, isOutput=False)
    out_d = nc.declare_dram_parameter("out", [s, D], f32, isOutput=True)
    if debug:
        dbg_kt = nc.declare_dram_parameter("dbg_kt", [128, s], bf16,
                                           isOutput=True)
        dbg_v4 = nc.declare_dram_parameter("dbg_v4", [KC, 128, HPC, 65], bf16,
                                           isOutput=True)
        dbg_ex = nc.declare_dram_parameter("dbg_ex", [128, HPC, QB], bf16,
                                           isOutput=True)

    with tile.TileContext(nc) as tc:
        import contextlib
        with contextlib.ExitStack() as ctx:
            wpool = ctx.enter_context(tc.tile_pool(name="w", bufs=1))
            xpool = ctx.enter_context(tc.tile_pool(name="x", bufs=4))
            kpool = ctx.enter_context(tc.tile_pool(name="kt", bufs=1))
            qpool = ctx.enter_context(tc.tile_pool(name="qt", bufs=NP))
            vtpool = ctx.enter_context(tc.tile_pool(name="vt", bufs=2))
            vpool = ctx.enter_context(tc.tile_pool(name="v4", bufs=kc_total))
            epool = ctx.enter_context(tc.tile_pool(name="ex", bufs=16))
            dpool = ctx.enter_context(tc.tile_pool(name="dn", bufs=2))
            cpool = ctx.enter_context(tc.tile_pool(name="csn", bufs=2))
            rpool = ctx.enter_context(tc.tile_pool(name="recb", bufs=2))
            opool = ctx.enter_context(tc.tile_pool(name="outs", bufs=3))
            # PSUM: stg 3x2 banks + ctx0/ctx1 = 8; projections (q-block 0)
            # ride the stg ring, normalize/out-proj ride the ctx banks
            stg = ctx.enter_context(tc.tile_pool(name="stg", bufs=3,
                                                 space="PSUM"))
            cp = ctx.enter_context(tc.tile_pool(name="cp", bufs=1,
                                                space="PSUM"))

            # ---- constants / weights ----
            wq_t = wpool.tile([128, D], bf16, tag="wq")
            wk_t = wpool.tile([128, D], bf16, tag="wk")
            wv_t = wpool.tile([128, D], bf16, tag="wv")
            wo_t = wpool.tile([128, D], f32r, tag="wo")
            bq_t = wpool.tile([128, 1], f32, tag="bq")
            bk_t = wpool.tile([128, 1], f32, tag="bk")
            bv_t = wpool.tile([128, 1], f32, tag="bv")
            ones_f = wpool.tile([65, 64], f32, tag="ones_f")
            ones_t = wpool.tile([65, 64], f32r, tag="ones")
            ident = wpool.tile([128, 128], f32, tag="ident")

            # exp table prewarm: tiny activation issued first so the ~2.7us
            # ACT_TABLE_LOAD overlaps the input DMAs
            warm_s = wpool.tile([1, 2], f32, tag="warm_s")
            warm_d = wpool.tile([1, 2], f32, tag="warm_d")
            nc.vector.memset(warm_s[:], 0.0)
            nc.scalar.activation(warm_d[:], warm_s[:], Exp, bias=0.0,
                                 scale=1.0)

            nc.sync.dma_start(wk_t[:], wk_d[:])
            nc.sync.dma_start(bk_t[:], bk_d[:])
            nc.sync.dma_start(wq_t[:], wq_d[:])
            nc.sync.dma_start(bq_t[:], bq_d[:])

            kT = kpool.tile([128, s], bf16, tag="kT")
            q_tiles = []
            v_tiles = [None] * kc_total

            def mm(out, lhsT, rhs, start, stop, tile_position=None):
                return nc.tensor.matmul(out, lhsT, rhs, start=start, stop=stop,
                                        tile_position=tile_position)

            def emit_xdma(b, split=1):
                xb = xpool.tile([128, DC, QB], bf16, tag="xb")
                step = DC // split
                for i in range(split):
                    nc.sync.dma_start(
                        xb[:, i * step:(i + 1) * step, :],
                        xT_d[b, :, i * step:(i + 1) * step, :])
                return xb

            def proj_accum(w_t, xb, ps):
                for c in range(DC):
                    mm(ps[:], w_t[:, c * 128:(c + 1) * 128], xb[:, c, :],
                       start=(c == 0), stop=(c == DC - 1))

            def emit_proj(b, xb):
                # k + q projections share one stage-ring tile
                ps = stg.tile([128, 2, QB], f32, tag="stage")
                proj_accum(wk_t, xb, ps[:, 0, :])
                nc.vector.tensor_scalar_add(kT[:, b * QB:(b + 1) * QB],
                                            ps[:, 0, :], bk_t[:])
                proj_accum(wq_t, xb, ps[:, 1, :])
                qb = qpool.tile([128, QB], bf16, tag="qT")
                nc.vector.tensor_scalar_add(qb[:], ps[:, 1, :], bq_t[:])
                q_tiles.append(qb)
                # v projection + PE transposes share a second ring tile
                ps = stg.tile([128, 2, QB], f32, tag="stage")
                proj_accum(wv_t, xb, ps[:, 0, :])
                vt = vtpool.tile([128, QB], f32, tag="vt")
                nc.vector.tensor_scalar_add(vt[:], ps[:, 0, :], bv_t[:])
                tp = ps[:, 1, :]
                for j in range(QB // 128):
                    nc.tensor.transpose(tp[:, j * 128:(j + 1) * 128],
                                        vt[:, j * 128:(j + 1) * 128],
                                        ident[:])
                for j in range(QB // 128):
                    kc = b * (QB // 128) + j
                    v4 = vpool.tile([128, HPC, 65], bf16, tag="v4")
                    nc.vector.tensor_copy(
                        v4[:, :, 0:64],
                        tp[:, j * 128:(j + 1) * 128]
                        .rearrange("p (h m) -> p h m", h=HPC))
                    nc.vector.memset(v4[:, :, 64:65], 1.0)
                    v_tiles[kc] = v4
                    if debug:
                        nc.sync.dma_start(dbg_v4[kc], v4[:])

            # ---- prologue ----
            xb0 = emit_xdma(0, split=8)
            nc.sync.dma_start(wv_t[:], wv_d[:])
            nc.sync.dma_start(bv_t[:], bv_d[:])
            nc.sync.dma_start(wo_t[:], wo_d[:].bitcast(f32r))
            nc.vector.memset(ones_f[:], 1.0)
            nc.vector.tensor_copy(ones_t[:], ones_f[:])
            make_identity(nc, ident[:])

            emit_proj(0, xb0)
            if debug:
                nc.sync.dma_start(dbg_kt[:], kT[:])
            pending_xb = {1: emit_xdma(1, split=2), 2: emit_xdma(2, split=2)}
            next_proj = 1

            scale = float(1.0 / np.sqrt(HD))

            # slot-stream staging: 3 (kc, h) score slots share one PSUM tile
            # and one FD=1536 ACTIVATE; slots span q-block boundaries
            GS = 2
            sst = {"tile": None, "fill": 0, "pend": []}
            stash = {}

            def fire_group():
                n = sst["fill"]
                if n == 0:
                    return
                ex = epool.tile([128, GS, QB], bf16, tag="ex")
                nc.scalar.activation(ex[:, 0:n, :], sst["tile"][:, 0:n, :],
                                     Exp, bias=0.0, scale=scale)
                if debug and not sst.get("dumped"):
                    nc.sync.dma_start(dbg_ex[:], ex[:, 0:2, :])
                    sst["dumped"] = True
                for (g, h, sidx) in sst["pend"]:
                    stash[(g, h)] = (ex, sidx)
                sst["tile"] = None
                sst["fill"] = 0
                sst["pend"] = []

            def push_slot(qb, g, kc, h):
                if sst["tile"] is None:
                    st_new = stg.tile([128, GS, QB], f32, tag="stage")
                    sst["tile"] = st_new
                sidx = sst["fill"]
                mm(sst["tile"][:, sidx, :],
                   kT[h * 64:(h + 1) * 64, kc * 128:(kc + 1) * 128],
                   qb[h * 64:(h + 1) * 64, :],
                   start=True, stop=True, tile_position=(h * 64, 0))
                sst["pend"].append((g, h, sidx))
                sst["fill"] += 1
                if sst["fill"] == GS:
                    fire_group()

            # ---- attention: rolling pipelined stream ----
            # scores/exp always run LAG groups ahead of attn@v, across
            # q-block boundaries; epilogue out-proj is deferred and drip-fed
            # into the next block's stream so the PE FIFO never bursts.
            LAG = 8
            TOT = NP * kc_total
            st_state = {"emitted": 0}
            pend_ops = []

            def emit_one_scores():
                g = st_state["emitted"]
                Qs, kcs = divmod(g, kc_total)
                if Qs == 0:
                    while next_proj[0] <= kcs // 4 and next_proj[0] < NP:
                        b = next_proj[0]
                        emit_proj(b, pending_xb.pop(b))
                        if b + 2 < NP:
                            pending_xb[b + 2] = emit_xdma(b + 2, split=2)
                        next_proj[0] += 1
                for h in range(HPC):
                    push_slot(q_tiles[Qs], g, kcs, h)
                st_state["emitted"] += 1

            def drain_op():
                if pend_ops:
                    pend_ops.pop(0)()

            def epilogue_a(Q, ctxp0, ctxp1):
                dn = dpool.tile([65, HPC, QB], f32r, tag="dn")
                nc.vector.tensor_copy(dn[64:65, 0, :], ctxp0[64:65, :])
                nc.vector.tensor_copy(dn[64:65, 1, :], ctxp1[64:65, :])
                rbs = stg.tile([128, HPC, QB], f32, tag="stage")
                mm(rbs[0:64, 0, :], ones_t[64:65, :], dn[64:65, 0, :],
                   start=True, stop=True)
                mm(rbs[0:64, 1, :], ones_t[64:65, :], dn[64:65, 1, :],
                   start=True, stop=True)
                rec = rpool.tile([64, HPC, QB], f32, tag="rec")
                nc.vector.reciprocal_approx_fast(rec[:, 0, :],
                                                 rbs[0:64, 0, :])
                nc.vector.reciprocal_approx_fast(rec[:, 1, :],
                                                 rbs[0:64, 1, :])
                csn = cpool.tile([128, QB], f32r, tag="csn")
                cst = cpool.tile([64, QB], f32r, tag="cst")
                for m in range(QB // 128):
                    sl = slice(m * 128, (m + 1) * 128)
                    nc.vector.tensor_mul(csn[0:64, sl], ctxp0[0:64, sl],
                                         rec[:, 0, sl])
                    nc.vector.tensor_mul(cst[:, sl], ctxp1[0:64, sl],
                                         rec[:, 1, sl])
                    nc.sync.dma_start(csn[64:128, sl], cst[:, sl])

                def op_use(chunk, csn=csn, Q=Q):
                    ops = stg.tile([128, GS, 512], f32, tag="stage")
                    for i, (m, nh) in enumerate(chunk):
                        sl = slice(m * 128, (m + 1) * 128)
                        mm(ops[:, i, :], csn[:, sl],
                           wo_t[:, nh * 512:(nh + 1) * 512],
                           start=True, stop=True)
                        ob = opool.tile([128, 512], f32, tag="ob")
                        nc.vector.tensor_copy(ob[:], ops[:, i, :])
                        nc.sync.dma_start(
                            out_d[Q * QB + m * 128:Q * QB + (m + 1) * 128,
                                  nh * 512:(nh + 1) * 512],
                            ob[:])

                allops = [(m, nh) for m in range(QB // 128)
                          for nh in range(D // 512)]
                for chunk in (allops[0:2], allops[2:4], allops[4:6],
                              allops[6:8]):
                    pend_ops.append(lambda c=chunk: op_use(c))

            next_proj = [next_proj]
            for _ in range(LAG):
                emit_one_scores()

            ctxp0 = ctxp1 = None
            for g in range(TOT):
                Q, kc = divmod(g, kc_total)
                if kc == 0:
                    ctxp0 = cp.tile([128, QB], f32, tag="ctx0")
                    ctxp1 = cp.tile([128, QB], f32, tag="ctx1")
                if st_state["emitted"] < TOT:
                    emit_one_scores()
                for h in range(HPC):
                    while (g, h) not in stash:
                        if st_state["emitted"] < TOT:
                            emit_one_scores()
                        else:
                            fire_group()
                    ex, sidx = stash.pop((g, h))
                    ctxp = ctxp0 if h == 0 else ctxp1
                    mm(ctxp[0:65, :], v_tiles[kc][:, h, :], ex[:, sidx, :],
                       start=(kc == 0), stop=(kc == kc_total - 1))
                if kc in (6, 10, 14, 18):
                    drain_op()
                if kc == kc_total - 1:
                    epilogue_a(Q, ctxp0, ctxp1)
            while pend_ops:
                drain_op()

    nc.compile()
    return nc


def _shard_inputs(x, wq, bq, wk, bk, wv, bv, wo, bo, s):
    import ml_dtypes
    bf16 = ml_dtypes.bfloat16

    # [D, s] -> contiguous per-block layout [s//512, 128, D//128, 512]
    xT2 = np.asarray(x, np.float32).reshape(s, D).T
    xT = np.ascontiguousarray(
        xT2.reshape(D // 128, 128, s // 512, 512).transpose(2, 1, 0, 3)
        .astype(bf16))

    def lhsT_layout(w, c):
        blk = np.asarray(w, np.float32)[:, c * 128:(c + 1) * 128]
        return np.ascontiguousarray(
            blk.reshape(DC, 128, 128).transpose(1, 0, 2).reshape(128, D)
            .astype(bf16))

    msk = np.zeros((2, 128), np.float32)
    msk[0, 0:64] = 1.0
    msk[1, 64:128] = 1.0

    in_maps = []
    for c in range(N_CORES):
        in_maps.append({
            "xT": xT,
            "wq": lhsT_layout(wq, c),
            "wk": lhsT_layout(wk, c),
            "wv": lhsT_layout(wv, c),
            "bq": np.ascontiguousarray(
                np.asarray(bq, np.float32)[c * 128:(c + 1) * 128, None]),
            "bk": np.ascontiguousarray(
                np.asarray(bk, np.float32)[c * 128:(c + 1) * 128, None]),
            "bv": np.ascontiguousarray(
                np.asarray(bv, np.float32)[c * 128:(c + 1) * 128, None]),
            "wo": np.ascontiguousarray(
                np.asarray(wo, np.float32)[c * 128:(c + 1) * 128, :]),
            "msk": msk,
        })
    return in_maps


def run(x, wq, bq, wk, bk, wv, bv, wo, bo, trace=False, s=S):
    global _LAST_EXEC_NS
    from concourse.bass_utils import run_bass_kernel_spmd

    if trace:
        _install_ntff_hook_shim()
    nc = _build(s)
    in_maps = _shard_inputs(x, wq, bq, wk, bk, wv, bv, wo, bo, s)
    res = run_bass_kernel_spmd(nc, in_maps, core_ids=list(range(N_CORES)),
                               trace=trace)
    _LAST_EXEC_NS = res.exec_time_ns
    out = res.results[0]["out"].astype(np.float64)
    for c in range(1, N_CORES):
        out += res.results[c]["out"]
    out += np.asarray(bo, np.float64)
    return out.astype(np.float32).reshape(1, s, D)


def kernel(x, wq, bq, wk, bk, wv, bv, wo, bo):
    trace = bool(os.environ.get("BASS_MHA_TRACE"))
    return run(x, wq, bq, wk, bk, wv, bv, wo, bo, trace=trace)


# revision 64
# speedup vs baseline: 1.1841x; 1.1841x over previous
"""Multi-head attention (B=1, S=4096, D=1024, H=16, Hd=64) on 8 Trainium2 cores.

Sharding: tensor-parallel over heads - 2 heads per core. Each core computes
q/k/v projections for its 2 heads (128 dims), flash-style attention without
max-subtraction (scores are ~N(0,1) after scaling so exp never overflows),
and a partial output projection with its 128 rows of wo. Host sums the 8
partial outputs and adds bo.

v2 design (ACT-roofline targeted):
  - k/q/v and the exp'd scores are bf16 on SBUF: score matmuls get FWL
    weight loads and the two K=64 head-matmuls are explicitly row-tiled
    (tile_position (0,0)/(64,0)) into different PSUM banks so they run
    concurrently in the PE array.
  - PSUM: 4 banks score staging (2 bufs x [128,2,512]), 2 banks ctx
    accumulators (per-head, 65 rows: 64 ctx dims + ones-column denominator),
    2 banks aux ring used by projection accumulation first and by the
    normalize-broadcast + out-projection afterwards, so the per-q-block
    epilogue never blocks the next q-block's accumulation.
  - V is transposed to [k, hd] layout by the DMA xbar (bf16) instead of PE
    transposes.
  - Projections for x-blocks 1..7 are software-pipelined into q-block 0's
    attention groups.
  - Normalizer broadcast is one K=2 masked matmul for both heads.
"""

import os
import sys
import types

import numpy as np

S = 4096
D = 1024
H = 16
HD = 64
N_CORES = 8
HPC = H // N_CORES  # heads per core = 2
DC = D // 128       # d-chunks = 8
QB = 512            # q block
KC = S // 128       # k chunks = 32

_LAST_EXEC_NS = None


def _install_ntff_hook_shim():
    if "antenv.axon_hooks" in sys.modules:
        return
    try:
        import antenv
        from trn_agent_boot.trn_boot import _ntff_profile_via_ctypes

        hook = _ntff_profile_via_ctypes("/opt/axon/libaxon_pjrt.so")
    except Exception:
        return
    mod = types.ModuleType("antenv.axon_hooks")
    _state = {"hook": hook}
    mod.get_axon_ntff_profile_hook = lambda: _state["hook"]
    mod.set_axon_ntff_profile_hook = lambda h: _state.update(hook=h)
    sys.modules["antenv.axon_hooks"] = mod
    antenv.axon_hooks = mod


def _build(s=S, debug=False):
    import concourse.bass as bass
    import concourse.mybir as mybir
    import concourse.tile as tile
    from concourse import bacc
    from concourse.masks import make_identity

    f32 = mybir.dt.float32
    f32r = mybir.dt.float32r
    bf16 = mybir.dt.bfloat16
    Exp = mybir.ActivationFunctionType.Exp

    kc_total = s // 128
    NP = s // QB            # x/q blocks = 8
    NKP = kc_total // 2     # kc pairs per q block = 16

    nc = bacc.Bacc("TRN2", target_bir_lowering=False, debug=False,
                   num_devices=N_CORES)

    xT_d = nc.declare_dram_parameter("xT", [NP, 128, DC, QB], bf16,
                                     isOutput=False)
    wq_d = nc.declare_dram_parameter("wq", [128, D], bf16, isOutput=False)
    wk_d = nc.declare_dram_parameter("wk", [128, D], bf16, isOutput=False)
    wv_d = nc.declare_dram_parameter("wv", [128, D], bf16, isOutput=False)
    bq_d = nc.declare_dram_parameter("bq", [128, 1], f32, isOutput=False)
    bk_d = nc.declare_dram_parameter("bk", [128, 1], f32, isOutput=False)
    bv_d = nc.declare_dram_parameter("bv", [128, 1], f32, isOutput=False)
    wo_d = nc.declare_dram_parameter("wo", [128, D], f32, isOutput=False)
    out_d = nc.declare_dram_parameter("out", [s, D], f32, isOutput=True)
    if debug:
        dbg_kt = nc.declare_dram_parameter("dbg_kt", [128, s], bf16,
                                           isOutput=True)
        dbg_v4 = nc.declare_dram_parameter("dbg_v4", [KC, 128, HPC, 65], bf16,
                                           isOutput=True)
        dbg_ex = nc.declare_dram_parameter("dbg_ex", [128, HPC, QB], bf16,
                                           isOutput=True)

    with tile.TileContext(nc) as tc:
        import contextlib
        with contextlib.ExitStack() as ctx:
            wpool = ctx.enter_context(tc.tile_pool(name="w", bufs=1))
            xpool = ctx.enter_context(tc.tile_pool(name="x", bufs=4))
            kpool = ctx.enter_context(tc.tile_pool(name="kt", bufs=1))
            qpool = ctx.enter_context(tc.tile_pool(name="qt", bufs=NP))
            vtpool = ctx.enter_context(tc.tile_pool(name="vt", bufs=2))
            vpool = ctx.enter_context(tc.tile_pool(name="v4", bufs=kc_total))
            epool = ctx.enter_context(tc.tile_pool(name="ex", bufs=14))
            dpool = ctx.enter_context(tc.tile_pool(name="dn", bufs=2))
            cpool = ctx.enter_context(tc.tile_pool(name="csn", bufs=2))
            rpool = ctx.enter_context(tc.tile_pool(name="recb", bufs=2))
            opool = ctx.enter_context(tc.tile_pool(name="outs", bufs=3))
            # PSUM: stg 3x2 banks + ctx0/ctx1 = 8; projections (q-block 0)
            # ride the stg ring, normalize/out-proj ride the ctx banks
            stg = ctx.enter_context(tc.tile_pool(name="stg", bufs=3,
                                                 space="PSUM"))
            cp = ctx.enter_context(tc.tile_pool(name="cp", bufs=1,
                                                space="PSUM"))

            # ---- constants / weights ----
            wq_t = wpool.tile([128, D], bf16, tag="wq")
            wk_t = wpool.tile([128, D], bf16, tag="wk")
            wv_t = wpool.tile([128, D], bf16, tag="wv")
            wo_t = wpool.tile([128, D], f32r, tag="wo")
            bq_t = wpool.tile([128, 1], f32, tag="bq")
            bk_t = wpool.tile([128, 1], f32, tag="bk")
            bv_t = wpool.tile([128, 1], f32, tag="bv")
            ones_f = wpool.tile([65, 64], f32, tag="ones_f")
            ones_t = wpool.tile([65, 64], f32r, tag="ones")
            ident = wpool.tile([128, 128], f32, tag="ident")

            # exp table prewarm: tiny activation issued first so the ~2.7us
            # ACT_TABLE_LOAD overlaps the input DMAs
            warm_s = wpool.tile([1, 2], f32, tag="warm_s")
            warm_d = wpool.tile([1, 2], f32, tag="warm_d")
            nc.vector.memset(warm_s[:], 0.0)
            nc.scalar.activation(warm_d[:], warm_s[:], Exp, bias=0.0,
                                 scale=1.0)

            nc.sync.dma_start(wk_t[:], wk_d[:])
            nc.sync.dma_start(bk_t[:], bk_d[:])
            nc.sync.dma_start(wq_t[:], wq_d[:])
            nc.sync.dma_start(bq_t[:], bq_d[:])

            kT = kpool.tile([128, s], bf16, tag="kT")
            q_tiles = []
            v_tiles = [None] * kc_total

            def mm(out, lhsT, rhs, start, stop, tile_position=None):
                return nc.tensor.matmul(out, lhsT, rhs, start=start, stop=stop,
                                        tile_position=tile_position)

            def emit_xdma(b, split=1):
                xb = xpool.tile([128, DC, QB], bf16, tag="xb")
                step = DC // split
                for i in range(split):
                    nc.sync.dma_start(
                        xb[:, i * step:(i + 1) * step, :],
                        xT_d[b, :, i * step:(i + 1) * step, :])
                return xb

            def proj_accum(w_t, xb, ps):
                for c in range(DC):
                    mm(ps[:], w_t[:, c * 128:(c + 1) * 128], xb[:, c, :],
                       start=(c == 0), stop=(c == DC - 1))

            def emit_proj(b, xb):
                # k + q projections share one stage-ring tile
                ps = stg.tile([128, 2, QB], f32, tag="stage")
                proj_accum(wk_t, xb, ps[:, 0, :])
                nc.vector.tensor_scalar_add(kT[:, b * QB:(b + 1) * QB],
                                            ps[:, 0, :], bk_t[:])
                proj_accum(wq_t, xb, ps[:, 1, :])
                qb = qpool.tile([128, QB], bf16, tag="qT")
                nc.vector.tensor_scalar_add(qb[:], ps[:, 1, :], bq_t[:])
                q_tiles.append(qb)
                # v projection + PE transposes share a second ring tile
                ps = stg.tile([128, 2, QB], f32, tag="stage")
                proj_accum(wv_t, xb, ps[:, 0, :])
                vt = vtpool.tile([128, QB], f32, tag="vt")
                nc.vector.tensor_scalar_add(vt[:], ps[:, 0, :], bv_t[:])
                tp = ps[:, 1, :]
                for j in range(QB // 128):
                    nc.tensor.transpose(tp[:, j * 128:(j + 1) * 128],
                                        vt[:, j * 128:(j + 1) * 128],
                                        ident[:])
                for j in range(QB // 128):
                    kc = b * (QB // 128) + j
                    v4 = vpool.tile([128, HPC, 65], bf16, tag="v4")
                    nc.vector.tensor_copy(
                        v4[:, :, 0:64],
                        tp[:, j * 128:(j + 1) * 128]
                        .rearrange("p (h m) -> p h m", h=HPC))
                    nc.vector.memset(v4[:, :, 64:65], 1.0)
                    v_tiles[kc] = v4
                    if debug:
                        nc.sync.dma_start(dbg_v4[kc], v4[:])

            # ---- prologue ----
            xb0 = emit_xdma(0, split=8)
            nc.sync.dma_start(wv_t[:], wv_d[:])
            nc.sync.dma_start(bv_t[:], bv_d[:])
            nc.sync.dma_start(wo_t[:], wo_d[:].bitcast(f32r))
            nc.vector.memset(ones_f[:], 1.0)
            nc.vector.tensor_copy(ones_t[:], ones_f[:])
            make_identity(nc, ident[:])

            emit_proj(0, xb0)
            if debug:
                nc.sync.dma_start(dbg_kt[:], kT[:])
            pending_xb = {1: emit_xdma(1, split=2), 2: emit_xdma(2, split=2)}
            next_proj = 1

            scale = float(1.0 / np.sqrt(HD))

            # slot-stream staging: 3 (kc, h) score slots share one PSUM tile
            # and one FD=1536 ACTIVATE; slots span q-block boundaries
            GS = 2
            sst = {"tile": None, "fill": 0, "pend": []}
            stash = {}

            def fire_group():
                n = sst["fill"]
                if n == 0:
                    return
                ex = epool.tile([128, GS, QB], bf16, tag="ex")
                nc.scalar.activation(ex[:, 0:n, :], sst["tile"][:, 0:n, :],
                                     Exp, bias=0.0, scale=scale)
                if debug and not sst.get("dumped"):
                    nc.sync.dma_start(dbg_ex[:], ex[:, 0:2, :])
                    sst["dumped"] = True
                for (g, h, sidx) in sst["pend"]:
                    stash[(g, h)] = (ex, sidx)
                sst["tile"] = None
                sst["fill"] = 0
                sst["pend"] = []

            def push_slot(qb, g, kc, h):
                if sst["tile"] is None:
                    st_new = stg.tile([128, GS, QB], f32, tag="stage")
                    sst["tile"] = st_new
                sidx = sst["fill"]
                mm(sst["tile"][:, sidx, :],
                   kT[h * 64:(h + 1) * 64, kc * 128:(kc + 1) * 128],
                   qb[h * 64:(h + 1) * 64, :],
                   start=True, stop=True, tile_position=(h * 64, 0))
                sst["pend"].append((g, h, sidx))
                sst["fill"] += 1
                if sst["fill"] == GS:
                    fire_group()

            # ---- attention: rolling pipelined stream ----
            # scores/exp always run LAG groups ahead of attn@v, across
            # q-block boundaries; epilogue out-proj is deferred and drip-fed
            # into the next block's stream so the PE FIFO never bursts.
            LAG = 8
            TOT = NP * kc_total
            st_state = {"emitted": 0}
            pend_ops = []

            def emit_one_scores():
                g = st_state["emitted"]
                Qs, kcs = divmod(g, kc_total)
                if Qs == 0:
                    while next_proj[0] <= kcs // 4 and next_proj[0] < NP:
                        b = next_proj[0]
                        emit_proj(b, pending_xb.pop(b))
                        if b + 2 < NP:
                            pending_xb[b + 2] = emit_xdma(b + 2, split=2)
                        next_proj[0] += 1
                for h in range(HPC):
                    push_slot(q_tiles[Qs], g, kcs, h)
                st_state["emitted"] += 1

            def drain_op():
                if pend_ops:
                    pend_ops.pop(0)()

            def epilogue_a(Q, ctxp0, ctxp1):
                dn = dpool.tile([65, HPC, QB], f32r, tag="dn")
                nc.vector.tensor_copy(dn[64:65, 0, :], ctxp0[64:65, :])
                nc.vector.tensor_copy(dn[64:65, 1, :], ctxp1[64:65, :])
                rbs = stg.tile([128, HPC, QB], f32, tag="stage")
                mm(rbs[0:64, 0, :], ones_t[64:65, :], dn[64:65, 0, :],
                   start=True, stop=True)
                mm(rbs[0:64, 1, :], ones_t[64:65, :], dn[64:65, 1, :],
                   start=True, stop=True)
                rec = rpool.tile([64, HPC, QB], f32, tag="rec")
                nc.vector.reciprocal_approx_fast(rec[:, 0, :],
                                                 rbs[0:64, 0, :])
                nc.vector.reciprocal_approx_fast(rec[:, 1, :],
                                                 rbs[0:64, 1, :])
                csn = cpool.tile([128, QB], f32r, tag="csn")
                cst = cpool.tile([64, QB], f32r, tag="cst")
                for m in range(QB // 128):
                    sl = slice(m * 128, (m + 1) * 128)
                    nc.vector.tensor_mul(csn[0:64, sl], ctxp0[0:64, sl],
                                         rec[:, 0, sl])
                    nc.vector.tensor_mul(cst[:, sl], ctxp1[0:64, sl],
                                         rec[:, 1, sl])
                    nc.sync.dma_start(csn[64:128, sl], cst[:, sl])

                def op_use(chunk, csn=csn, Q=Q):
                    ops = stg.tile([128, GS, 512], f32, tag="stage")
                    for i, (m, nh) in enumerate(chunk):
                        sl = slice(m * 128, (m + 1) * 128)
                        mm(ops[:, i, :], csn[:, sl],
                           wo_t[:, nh * 512:(nh + 1) * 512],
                           start=True, stop=True)
                        ob = opool.tile([128, 512], f32, tag="ob")
                        nc.vector.tensor_copy(ob[:], ops[:, i, :])
                        nc.sync.dma_start(
                            out_d[Q * QB + m * 128:Q * QB + (m + 1) * 128,
                                  nh * 512:(nh + 1) * 512],
                            ob[:])

                allops = [(m, nh) for m in range(QB // 128)
                          for nh in range(D // 512)]
                for chunk in (allops[0:2], allops[2:4], allops[4:6],
                              allops[6:8]):
                    pend_ops.append(lambda c=chunk: op_use(c))

            next_proj = [next_proj]
            for _ in range(LAG):
                emit_one_scores()

            ctxp0 = ctxp1 = None
            for g in range(TOT):
                Q, kc = divmod(g, kc_total)
                if kc == 0:
                    ctxp0 = cp.tile([128, QB], f32, tag="ctx0")
                    ctxp1 = cp.tile([128, QB], f32, tag="ctx1")
                if st_state["emitted"] < TOT:
                    emit_one_scores()
                for h in range(HPC):
                    while (g, h) not in stash:
                        if st_state["emitted"] < TOT:
                            emit_one_scores()
                        else:
                            fire_group()
                    ex, sidx = stash.pop((g, h))
                    ctxp = ctxp0 if h == 0 else ctxp1
                    mm(ctxp[0:65, :], v_tiles[kc][:, h, :], ex[:, sidx, :],
                       start=(kc == 0), stop=(kc == kc_total - 1))
                if kc in (4, 8, 12, 16):
                    drain_op()
                if kc == kc_total - 1:
                    epilogue_a(Q, ctxp0, ctxp1)
            while pend_ops:
                drain_op()

    nc.compile()
    return nc


def _shard_inputs(x, wq, bq, wk, bk, wv, bv, wo, bo, s):
    import ml_dtypes
    bf16 = ml_dtypes.bfloat16

    # [D, s] -> contiguous per-block layout [s//512, 128, D//128, 512]
    xT2 = np.asarray(x, np.float32).reshape(s, D).T
    xT = np.ascontiguousarray(
        xT2.reshape(D // 128, 128, s // 512, 512).transpose(2, 1, 0, 3)
        .astype(bf16))

    def lhsT_layout(w, c):
        blk = np.asarray(w, np.float32)[:, c * 128:(c + 1) * 128]
        return np.ascontiguousarray(
            blk.reshape(DC, 128, 128).transpose(1, 0, 2).reshape(128, D)
            .astype(bf16))

    msk = np.zeros((2, 128), np.float32)
    msk[0, 0:64] = 1.0
    msk[1, 64:128] = 1.0

    in_maps = []
    for c in range(N_CORES):
        in_maps.append({
            "xT": xT,
            "wq": lhsT_layout(wq, c),
            "wk": lhsT_layout(wk, c),
            "wv": lhsT_layout(wv, c),
            "bq": np.ascontiguousarray(
                np.asarray(bq, np.float32)[c * 128:(c + 1) * 128, None]),
            "bk": np.ascontiguousarray(
                np.asarray(bk, np.float32)[c * 128:(c + 1) * 128, None]),
            "bv": np.ascontiguousarray(
                np.asarray(bv, np.float32)[c * 128:(c + 1) * 128, None]),
            "wo": np.ascontiguousarray(
                np.asarray(wo, np.float32)[c * 128:(c + 1) * 128, :]),
            "msk": msk,
        })
    return in_maps


def run(x, wq, bq, wk, bk, wv, bv, wo, bo, trace=False, s=S):
    global _LAST_EXEC_NS
    from concourse.bass_utils import run_bass_kernel_spmd

    if trace:
        _install_ntff_hook_shim()
    nc = _build(s)
    in_maps = _shard_inputs(x, wq, bq, wk, bk, wv, bv, wo, bo, s)
    res = run_bass_kernel_spmd(nc, in_maps, core_ids=list(range(N_CORES)),
                               trace=trace)
    _LAST_EXEC_NS = res.exec_time_ns
    out = res.results[0]["out"].astype(np.float64)
    for c in range(1, N_CORES):
        out += res.results[c]["out"]
    out += np.asarray(bo, np.float64)
    return out.astype(np.float32).reshape(1, s, D)


def kernel(x, wq, bq, wk, bk, wv, bv, wo, bo):
    trace = bool(os.environ.get("BASS_MHA_TRACE"))
    return run(x, wq, bq, wk, bk, wv, bv, wo, bo, trace=trace)
